# revision 13
# baseline (speedup 1.0000x reference)
"""Causal multi-head attention forward on 8 Trainium2 NeuronCores.

Problem: nn_CoreAttention (SQ=SK=2048, B=2, NP=16 heads, HN=128, fp32).

Sharding: 32 (batch, head) pairs split 4 per core (tensor-parallel over
heads, data-parallel over batch). No collectives.

Per (b, n) pair, in transposed score orientation (sk on partitions):
    scoresT[sk, sq] = (K Q^T)                 (PE matmul, fp16 in, hn contracted)
    expT = exp(scoresT * 1/sqrt(HN) + mask)   (ScalarE exp, except whole chunks
                                               assigned to a DVE exp2 bit-trick:
                                               i16 = rne_sat(c*x + b) bitcast
                                               fp16; masked cols saturate to
                                               -32768 = fp16 -0.0)
    ctx_aug[sq, hn+1] = expT^T @ [V | 1]      (PE matmul, sk contracted;
                                               col hn holds the softmax denom)
    ctx = ctx_aug[:, :hn] * 1/ctx_aug[:, hn]  (DVE reciprocal + scale, fp16 out)

v5 structure (vs the 94.5us v2 baseline):
  - exp work is split ACT/DVE at WHOLE-CHUNK granularity (chunks in
    ATT_DVE_CHUNKS go to the DVE bit-trick).  Whole-row approximation makes
    the softmax ratio cancel most of the exp2 interpolation error: measured
    ~1e-3 final vs ~1e-2 for column-sliced splits.
  - outputs stored + DMA'd as fp16 (host casts back to fp32).
  - PE warm-up matmuls release the HAM clock gate during the initial DMA wait.
  - slot 0 walks chunks ASCENDING so k-tile demand matches DMA arrival; other
    slots walk descending so the kernel tail is the smallest chunk.
  - last-slot output DMAs alternate between the sync and scalar queues.
"""

import math
import numpy as np
from contextlib import ExitStack

import concourse.bacc as bacc
import concourse.tile as tile
from concourse import mybir

SQ, SK, B, NP, HN = 2048, 2048, 2, 16, 128
N_CORES = 8
SLOTS_PER_CORE = 4
P = 128
CHUNK = 256
N_CHUNKS = SQ // CHUNK      # 8
N_SK_TILES = SK // P        # 16
NEG = -60000.0

import os
GROUP = int(os.environ.get("ATT_GROUP", "4"))          # 256-col units per group
SC_BUFS = int(os.environ.get("ATT_SC_BUFS", "3"))
CX_BUFS = int(os.environ.get("ATT_CX_BUFS", "2"))
E_BUFS = int(os.environ.get("ATT_E_BUFS", "8"))
DVE_CHUNKS = tuple(
    int(x) for x in os.environ.get("ATT_DVE_CHUNKS", "3,5").split(",") if x != ""
)
MU = float(os.environ.get("ATT_MU", "40.0"))
WARM_MMS = int(os.environ.get("ATT_WARM_MMS", "40"))

F32 = mybir.dt.float32
F16 = mybir.dt.float16
I16 = mybir.dt.int16

INV_NORM = 1.0 / math.sqrt(HN)
C_MUL = INV_NORM * math.log2(math.e) * 1024.0
B_ADD = 15360.0 + MU


def _build_program():
    nc = bacc.Bacc()

    qT_d = nc.declare_dram_parameter("qT", [SLOTS_PER_CORE, P, SQ], F16, isOutput=False)
    kT_d = nc.declare_dram_parameter("kT", [SLOTS_PER_CORE, P, SK], F16, isOutput=False)
    v_d = nc.declare_dram_parameter(
        "v_aug", [SLOTS_PER_CORE, P, N_SK_TILES * (HN + 1)], F16, isOutput=False
    )
    # triT[p, c] = NEG if p < c else 0; ident2 = [I | I]
    triT_d = nc.declare_dram_parameter("triT", [P, P], F16, isOutput=False)
    id2_d = nc.declare_dram_parameter("ident2", [P, 2 * P], F16, isOutput=False)
    out_d = nc.declare_dram_parameter(
        "out", [SLOTS_PER_CORE, 4, P, 4 * HN], F16, isOutput=True
    )

    with tile.TileContext(nc) as tc, ExitStack() as ctx:
        qk_pool = ctx.enter_context(tc.tile_pool(name="qk", bufs=2))
        v_pool = ctx.enter_context(tc.tile_pool(name="v", bufs=2))
        m_pool = ctx.enter_context(tc.tile_pool(name="m", bufs=1))
        e_pool = ctx.enter_context(tc.tile_pool(name="e", bufs=E_BUFS))
        o_pool = ctx.enter_context(tc.tile_pool(name="o", bufs=4))
        r_pool = ctx.enter_context(tc.tile_pool(name="r", bufs=4))
        sc_ps = ctx.enter_context(tc.tile_pool(name="sc", bufs=SC_BUFS, space="PSUM"))
        cx_ps = ctx.enter_context(tc.tile_pool(name="cx", bufs=CX_BUFS, space="PSUM"))

        triT_sb = m_pool.tile([P, P], F16, tag="triT")
        nc.scalar.dma_start(triT_sb[:], triT_d[:])
        id2_sb = m_pool.tile([P, 2 * P], F16, tag="id2")

        # touch Exp immediately so the ACT table loads during the initial DMAs
        warm_in = m_pool.tile([P, 1], F32, tag="warm_in")
        nc.vector.memset(warm_in[:], 0.0)
        warm_out = m_pool.tile([P, 1], F32, tag="warm_out")
        nc.scalar.activation(
            warm_out[:], warm_in[:], mybir.ActivationFunctionType.Exp
        )

        # ---- PE warm-up: release the HAM clock gate during the DMA wait ----
        if WARM_MMS:
            wm16 = m_pool.tile([P, 64], F16, tag="wm16")
            nc.vector.memset(wm16[:], 0.0)
            warm_sc = sc_ps.tile([P, GROUP * CHUNK], F32, tag="scores", name="warm")
            for _ in range(WARM_MMS):
                nc.tensor.matmul(
                    warm_sc[0:64, 0:64], wm16[:, 0:64], wm16[:, 0:64],
                    start=True, stop=True,
                )

        # ---- per-slot input loading -------------------------------------
        def load_slot(slot):
            """Returns (kslice, qchunk, vslice) accessor fns for this slot."""
            if slot == 0:
                # slot 0 walks chunks ASCENDING: chunk c needs k tiles
                # j <= 2c+1 and q chunk c, so demand tracks DMA arrival.
                kts = [
                    qk_pool.tile([P, 4 * P], F16, tag=f"k{pc}", name=f"k{pc}")
                    for pc in range(4)
                ]
                qts = [
                    qk_pool.tile([P, CHUNK], F16, tag=f"q{pc}", name=f"q{pc}")
                    for pc in range(N_CHUNKS)
                ]
                vts = [
                    v_pool.tile([P, 8 * (HN + 1)], F16, tag=f"v{pc}", name=f"v{pc}")
                    for pc in range(2)
                ]
                nc.sync.dma_start(kts[0][:, 0:256], kT_d[slot][:, 0:256])
                nc.scalar.dma_start(qts[7][:], qT_d[slot][:, 7 * CHUNK : 8 * CHUNK])
                nc.sync.dma_start(kts[0][:, 256:512], kT_d[slot][:, 256:512])
                nc.scalar.dma_start(kts[1][:], kT_d[slot][:, 512:1024])
                nc.sync.dma_start(kts[2][:], kT_d[slot][:, 1024:1536])
                nc.scalar.dma_start(id2_sb[:], id2_d[:])
                nc.sync.dma_start(kts[3][:], kT_d[slot][:, 1536:2048])
                nc.scalar.dma_start(qts[6][:], qT_d[slot][:, 6 * CHUNK : 7 * CHUNK])
                for pc in range(2):
                    nc.scalar.dma_start(
                        vts[pc][:],
                        v_d[slot][:, pc * 8 * (HN + 1) : (pc + 1) * 8 * (HN + 1)],
                    )
                for pc in range(N_CHUNKS - 3, -1, -1):
                    nc.sync.dma_start(
                        qts[pc][:], qT_d[slot][:, pc * CHUNK : (pc + 1) * CHUNK]
                    )
                kslice = lambda j: kts[j // 4][:, (j % 4) * P : (j % 4 + 1) * P]
                qchunk = lambda ci: qts[ci][:]
                vslice = lambda j: vts[j // 8][
                    :, (j % 8) * (HN + 1) : (j % 8 + 1) * (HN + 1)
                ]
            else:
                kt = qk_pool.tile([P, SK], F16, tag="k")
                nc.sync.dma_start(kt[:], kT_d[slot])
                qt = qk_pool.tile([P, SQ], F16, tag="q")
                nc.sync.dma_start(qt[:], qT_d[slot])
                vt = v_pool.tile([P, N_SK_TILES * (HN + 1)], F16, tag="v")
                nc.sync.dma_start(vt[:], v_d[slot])
                kslice = lambda j: kt[:, j * P : (j + 1) * P]
                qchunk = lambda ci: qt[:, ci * CHUNK : (ci + 1) * CHUNK]
                vslice = lambda j: vt[:, j * (HN + 1) : (j + 1) * (HN + 1)]
            return kslice, qchunk, vslice

        # ---- score packer (within a slot): QK blocks stream into shared
        # PSUM groups; at flush, consecutive same-engine chunk spans are
        # exp'd by one op each (ACT exp, or DVE exp2 bit-trick for chunks
        # in DVE_CHUNKS); flushed at slot boundaries
        CAP = GROUP * CHUNK
        etmap = {}
        packer = {"sc": None, "fill": 0, "entries": []}

        def flush_packer():
            if packer["sc"] is None or packer["fill"] == 0:
                return
            fill = packer["fill"]
            et = e_pool.tile([P, CAP], F16, tag="expT", name="et")
            # group entries into consecutive same-engine spans; the 128-col
            # pad after each diagonal block breaks adjacency so pads are
            # never exp'd (they are never written or read)
            spans = []   # (engine, start, end)
            for (slot, ci, j), off, w in packer["entries"]:
                eng = "dve" if ci in DVE_CHUNKS else "act"
                if spans and spans[-1][0] == eng and spans[-1][2] == off:
                    spans[-1][2] = off + w
                else:
                    spans.append([eng, off, off + w])
            for eng, s, e in sorted(spans, key=lambda x: x[0] != "dve"):
                if eng == "dve":
                    nc.vector.tensor_scalar(
                        et.bitcast(I16)[:, s:e],
                        packer["sc"][:, s:e],
                        C_MUL, B_ADD,
                        mybir.AluOpType.mult, mybir.AluOpType.add,
                    )
                else:
                    nc.scalar.activation(
                        et[:, s:e], packer["sc"][:, s:e],
                        mybir.ActivationFunctionType.Exp,
                        scale=INV_NORM,
                    )
            for key, off, w in packer["entries"]:
                etmap[key] = (et, off)
            packer["sc"] = None
            packer["fill"] = 0
            packer["entries"] = []

        def emit_qk(slot, slot_io, ci):
            kslice, qchunk, _ = slot_io
            diag = 2 * ci + 1
            for j in range(2 * ci + 2):      # ascending; diagonal j last
                w = P if j == diag else CHUNK
                w_pad = CHUNK                  # diag padded to 256 so every
                                               # block stays 256-aligned and no
                                               # matmul crosses a PSUM bank
                if packer["sc"] is None or packer["fill"] + w_pad > CAP:
                    flush_packer()
                if packer["sc"] is None:
                    packer["sc"] = sc_ps.tile(
                        [P, CAP], F32, tag="scores", name="sc"
                    )
                sc, co = packer["sc"], packer["fill"]
                nc.tensor.matmul(
                    sc[:, co : co + w], kslice(j), qchunk(ci)[:, 0:w],
                    start=True, stop=True,
                )
                # causal mask on the PE: sc[m, n] += triT[n%128, m].
                # Must directly follow its QK matmul — start=False
                # continues only the most recent accumulation group.
                if j == diag:
                    nc.tensor.matmul(
                        sc[:, co : co + P], triT_sb[:], id2_sb[:, 0:P],
                        start=False, stop=True, skip_group_check=True,
                    )
                elif j == diag - 1:
                    nc.tensor.matmul(
                        sc[:, co + P : co + 2 * P], triT_sb[:], id2_sb[:, 0:P],
                        start=False, stop=True, skip_group_check=True,
                    )
                packer["entries"].append(((slot, ci, j), co, w))
                packer["fill"] = co + w_pad

        # ---- emit one chunk's PV + normalize + (maybe) out DMA ----------
        def emit_pv(slot, slot_io, ci, oq_tiles, done_quarters):
            _, _, vslice = slot_io
            exp_tiles = {j: etmap[(slot, ci, j)] for j in range(2 * ci + 2)}
            # one PSUM tile holds both context vectors of the chunk:
            # i_lo at cols [0,129), i_hi at cols [129,258)
            cx = cx_ps.tile([P, 2 * (HN + 1)], F32, tag="ctx")
            for i in (2 * ci + 1, 2 * ci):   # i_hi (first half of chunk), i_lo
                off = 0 if i == 2 * ci + 1 else P
                base = (HN + 1) if i == 2 * ci + 1 else 0
                pv_js = list(range(i + 1))
                for idx, j in enumerate(pv_js):
                    et, co = exp_tiles[j]
                    nc.tensor.matmul(
                        cx[:, base : base + HN + 1],
                        et[:, co + off : co + off + P], vslice(j),
                        start=(idx == 0), stop=(idx == len(pv_js) - 1),
                    )
            recip = r_pool.tile([P, 2], F32, tag="recip")
            nc.vector.reciprocal(
                recip[:], cx[:, HN : 2 * HN + 2 : HN + 1]
            )
            qt_idx = (2 * ci) // 4
            if qt_idx not in oq_tiles:
                oq_tiles[qt_idx] = o_pool.tile(
                    [P, 4 * HN], F16, tag="oq", name="oq"
                )
            ot = oq_tiles[qt_idx]
            col = (2 * ci % 4) * HN          # i_lo column; i_hi is the next one
            nc.vector.tensor_mul(
                ot[:, col : col + 2 * HN].rearrange("p (s c) -> p s c", s=2),
                cx[:].rearrange("p (s c) -> p s c", s=2)[:, :, 0:HN],
                recip[:].rearrange("p (s c) -> p s c", c=1).broadcast_to(
                    [P, 2, HN]
                ),
            )
            if slot == SLOTS_PER_CORE - 1:
                # last slot: ship each chunk's half-quarter as soon as it is
                # normalized, alternating queues so the final DMAs overlap
                h = ci % 2
                eng = nc.sync if ci % 2 == 0 else nc.scalar
                eng.dma_start(
                    out_d[slot, qt_idx][:, h * 2 * HN : (h + 1) * 2 * HN],
                    ot[:, h * 2 * HN : (h + 1) * 2 * HN],
                )
            else:
                done_quarters.setdefault(qt_idx, set()).add(ci)
                if len(done_quarters[qt_idx]) == 2:
                    nc.sync.dma_start(out_d[slot, qt_idx], oq_tiles[qt_idx][:])

        # ---- main schedule: PV runs as soon as its exp tiles exist ------
        pvq = []  # [(slot, slot_io, ci, oq_tiles, done_quarters)]

        def drain_pv(final=False):
            # keep one chunk pending (unless final) so PV trails the QK
            # stream; a chunk is ready once its diagonal block has been exp'd
            while pvq and (final or len(pvq) >= 2):
                slot, slot_io, ci, oq, dq = pvq[0]
                if (slot, ci, 2 * ci + 1) not in etmap:
                    return
                pvq.pop(0)
                emit_pv(slot, slot_io, ci, oq, dq)

        slot_state = {}
        for slot in range(SLOTS_PER_CORE):
            slot_io = load_slot(slot)
            slot_state[slot] = ({}, {})  # oq_tiles, done_quarters
            for ci in range(N_CHUNKS - 1, -1, -1):
                emit_qk(slot, slot_io, ci)
                if ci in DVE_CHUNKS:
                    # flush now so the DVE span starts draining its PSUM
                    # group immediately instead of waiting for the group
                    # to fill with the next chunk's blocks
                    flush_packer()
                oq, dq = slot_state[slot]
                pvq.append((slot, slot_io, ci, oq, dq))
                drain_pv()
            flush_packer()   # keep exp tiles slot-local
            drain_pv()
        drain_pv(final=True)
        assert not pvq

    nc.compile()
    return nc


_cache = {}


def _get_program(mask: np.ndarray):
    # this kernel is specialized to the standard causal mask
    m = np.asarray(mask)
    causal = np.triu(np.ones((SQ, SK), dtype=bool), k=1)
    for b in range(B):
        if not np.array_equal(m[b, 0], causal):
            raise ValueError("kernel specialized to causal attention mask")
    if "nc" not in _cache:
        _cache["nc"] = _build_program()
    return _cache["nc"]


def _core_slots(c):
    return [(0, 2 * c), (0, 2 * c + 1), (1, 2 * c), (1, 2 * c + 1)]


def prepare(query_layer, key_layer, value_layer, attention_mask):
    q = np.asarray(query_layer)
    k = np.asarray(key_layer)
    v = np.asarray(value_layer)
    nc = _get_program(np.asarray(attention_mask))

    # qT with the two 128-col tiles of each 256 chunk swapped:
    # sbuf layout col (256*ci + [0..255]) = sq (256*ci + [128..255, 0..127])
    q16 = q.astype(np.float16)                      # [SQ, B, NP, HN]
    qv = q16.reshape(N_CHUNKS, 2, P, B, NP, HN)[:, ::-1]   # swap tile pairs
    qT_all = np.ascontiguousarray(qv.transpose(3, 4, 5, 0, 1, 2)).reshape(
        B, NP, HN, SQ
    )
    k16 = k.astype(np.float16)
    kT_all = np.ascontiguousarray(k16.transpose(1, 2, 3, 0))  # [B, NP, HN, SK]

    v5 = v.reshape(N_SK_TILES, P, B, NP, HN).transpose(2, 3, 1, 0, 4)
    v_aug_all = np.empty((B, NP, P, N_SK_TILES, HN + 1), dtype=np.float16)
    v_aug_all[..., :HN] = v5
    v_aug_all[..., HN] = 1.0
    v_aug_all = v_aug_all.reshape(B, NP, P, N_SK_TILES * (HN + 1))

    # mask-matmul constants: sc[m, n] += sum_p triT[p, m] * ident2[p, n]
    #   = triT[n%128, m]  which must be NEG where (n%128) < m
    triT = np.where(
        np.arange(P)[:, None] < np.arange(P)[None, :], NEG, 0.0
    ).astype(np.float16)                            # triT[p, c] = NEG if p < c
    ident2 = np.concatenate([np.eye(P), np.eye(P)], axis=1).astype(np.float16)

    in_maps = []
    for c in range(N_CORES):
        slots = _core_slots(c)
        im = {
            "qT": np.ascontiguousarray(np.stack([qT_all[b, n] for b, n in slots])),
            "kT": np.ascontiguousarray(np.stack([kT_all[b, n] for b, n in slots])),
            "v_aug": np.ascontiguousarray(
                np.stack([v_aug_all[b, n] for b, n in slots])
            ),
            "triT": triT,
            "ident2": ident2,
        }
        in_maps.append(im)
    return nc, in_maps


def assemble(results):
    """Gather per-core 'out' arrays into the full [SQ, B, NP*HN] output."""
    full = np.empty((SQ, B, NP * HN), dtype=np.float32)
    for c in range(N_CORES):
        o = results[c]["out"]  # [4, 4, 128, 512] fp16
        for s, (b, n) in enumerate(_core_slots(c)):
            ctx = (
                o[s].reshape(4, P, 4, HN).transpose(0, 2, 1, 3).reshape(SQ, HN)
            )
            full[:, b, n * HN : (n + 1) * HN] = ctx.astype(np.float32)
    return full


def kernel(query_layer, key_layer, value_layer, attention_mask):
    from concourse.bass_utils import run_bass_kernel_spmd

    nc, in_maps = prepare(query_layer, key_layer, value_layer, attention_mask)
    res = run_bass_kernel_spmd(nc, in_maps, list(range(N_CORES)))
    return assemble(res.results)


# revision 14
# speedup vs baseline: 1.2744x; 1.2744x over previous
"""Causal multi-head attention forward on 8 Trainium2 NeuronCores.

Problem: nn_CoreAttention (SQ=SK=2048, B=2, NP=16 heads, HN=128, fp32).

Sharding: 32 (batch, head) pairs split 4 per core (tensor-parallel over
heads, data-parallel over batch). No collectives.

Per (b, n) pair, in transposed score orientation (sk on partitions):
    scoresT[sk, sq] = (K Q^T)                 (PE matmul, fp16 in, hn contracted)
    expT = exp(scoresT * 1/sqrt(HN) + mask)   (ScalarE exp, except whole chunks
                                               assigned to a DVE exp2 bit-trick:
                                               i16 = rne_sat(c*x + b) bitcast
                                               fp16; masked cols saturate to
                                               -32768 = fp16 -0.0)
    ctx_aug[sq, hn+1] = expT^T @ [V | 1]      (PE matmul, sk contracted;
                                               col hn holds the softmax denom)
    ctx = ctx_aug[:, :hn] * 1/ctx_aug[:, hn]  (DVE reciprocal + scale, fp16 out)

v5 structure (vs the 94.5us v2 baseline):
  - exp work is split ACT/DVE at WHOLE-CHUNK granularity (chunks in
    ATT_DVE_CHUNKS go to the DVE bit-trick).  Whole-row approximation makes
    the softmax ratio cancel most of the exp2 interpolation error: measured
    ~1e-3 final vs ~1e-2 for column-sliced splits.
  - outputs stored + DMA'd as fp16 (host casts back to fp32).
  - PE warm-up matmuls release the HAM clock gate during the initial DMA wait.
  - slot 0 walks chunks ASCENDING so k-tile demand matches DMA arrival; other
    slots walk descending so the kernel tail is the smallest chunk.
  - last-slot output DMAs alternate between the sync and scalar queues.
"""

import math
import numpy as np
from contextlib import ExitStack

import concourse.bacc as bacc
import concourse.tile as tile
from concourse import mybir

SQ, SK, B, NP, HN = 2048, 2048, 2, 16, 128
N_CORES = 8
SLOTS_PER_CORE = 4
P = 128
CHUNK = 256
N_CHUNKS = SQ // CHUNK      # 8
N_SK_TILES = SK // P        # 16
NEG = -60000.0

import os
GROUP = int(os.environ.get("ATT_GROUP", "4"))          # 256-col units per group
SC_BUFS = int(os.environ.get("ATT_SC_BUFS", "3"))
CX_BUFS = int(os.environ.get("ATT_CX_BUFS", "2"))
E_BUFS = int(os.environ.get("ATT_E_BUFS", "8"))
DVE_CHUNKS = tuple(
    int(x) for x in os.environ.get("ATT_DVE_CHUNKS", "3,5").split(",") if x != ""
)
MU = float(os.environ.get("ATT_MU", "40.0"))
WARM_MMS = int(os.environ.get("ATT_WARM_MMS", "40"))

F32 = mybir.dt.float32
F16 = mybir.dt.float16
I16 = mybir.dt.int16

INV_NORM = 1.0 / math.sqrt(HN)
C_MUL = INV_NORM * math.log2(math.e) * 1024.0
B_ADD = 15360.0 + MU


def _build_program():
    nc = bacc.Bacc()

    qT_d = nc.declare_dram_parameter("qT", [SLOTS_PER_CORE, P, SQ], F16, isOutput=False)
    kT_d = nc.declare_dram_parameter("kT", [SLOTS_PER_CORE, P, SK], F16, isOutput=False)
    v_d = nc.declare_dram_parameter(
        "v_aug", [SLOTS_PER_CORE, P, N_SK_TILES * (HN + 1)], F16, isOutput=False
    )
    # triT[p, c] = NEG if p < c else 0; ident2 = [I | I]
    triT_d = nc.declare_dram_parameter("triT", [P, P], F16, isOutput=False)
    id2_d = nc.declare_dram_parameter("ident2", [P, 2 * P], F16, isOutput=False)
    out_d = nc.declare_dram_parameter(
        "out", [SLOTS_PER_CORE, 4, P, 4 * HN], F16, isOutput=True
    )

    with tile.TileContext(nc) as tc, ExitStack() as ctx:
        qk_pool = ctx.enter_context(tc.tile_pool(name="qk", bufs=2))
        v_pool = ctx.enter_context(tc.tile_pool(name="v", bufs=2))
        m_pool = ctx.enter_context(tc.tile_pool(name="m", bufs=1))
        e_pool = ctx.enter_context(tc.tile_pool(name="e", bufs=E_BUFS))
        o_pool = ctx.enter_context(tc.tile_pool(name="o", bufs=4))
        r_pool = ctx.enter_context(tc.tile_pool(name="r", bufs=4))
        sc_ps = ctx.enter_context(tc.tile_pool(name="sc", bufs=SC_BUFS, space="PSUM"))
        cx_ps = ctx.enter_context(tc.tile_pool(name="cx", bufs=CX_BUFS, space="PSUM"))

        triT_sb = m_pool.tile([P, P], F16, tag="triT")
        nc.scalar.dma_start(triT_sb[:], triT_d[:])
        id2_sb = m_pool.tile([P, 2 * P], F16, tag="id2")

        # touch Exp immediately so the ACT table loads during the initial DMAs
        warm_in = m_pool.tile([P, 1], F32, tag="warm_in")
        nc.vector.memset(warm_in[:], 0.0)
        warm_out = m_pool.tile([P, 1], F32, tag="warm_out")
        nc.scalar.activation(
            warm_out[:], warm_in[:], mybir.ActivationFunctionType.Exp
        )

        # ---- PE warm-up: release the HAM clock gate during the DMA wait ----
        if WARM_MMS:
            wm16 = m_pool.tile([P, 64], F16, tag="wm16")
            nc.vector.memset(wm16[:], 0.0)
            warm_sc = sc_ps.tile([P, GROUP * CHUNK], F32, tag="scores", name="warm")
            for _ in range(WARM_MMS):
                nc.tensor.matmul(
                    warm_sc[0:64, 0:64], wm16[:, 0:64], wm16[:, 0:64],
                    start=True, stop=True,
                )

        # ---- per-slot input loading -------------------------------------
        def load_slot(slot):
            """Returns (kslice, qchunk, vslice) accessor fns for this slot."""
            if slot == 0:
                # slot 0 walks chunks ASCENDING: chunk c needs k tiles
                # j <= 2c+1 and q chunk c, so demand tracks DMA arrival.
                kts = [
                    qk_pool.tile([P, 4 * P], F16, tag=f"k{pc}", name=f"k{pc}")
                    for pc in range(4)
                ]
                qts = [
                    qk_pool.tile([P, CHUNK], F16, tag=f"q{pc}", name=f"q{pc}")
                    for pc in range(N_CHUNKS)
                ]
                vts = [
                    v_pool.tile([P, 8 * (HN + 1)], F16, tag=f"v{pc}", name=f"v{pc}")
                    for pc in range(2)
                ]
                nc.sync.dma_start(kts[0][:, 0:256], kT_d[slot][:, 0:256])
                nc.scalar.dma_start(qts[7][:], qT_d[slot][:, 7 * CHUNK : 8 * CHUNK])
                nc.sync.dma_start(kts[0][:, 256:512], kT_d[slot][:, 256:512])
                nc.scalar.dma_start(kts[1][:], kT_d[slot][:, 512:1024])
                nc.sync.dma_start(kts[2][:], kT_d[slot][:, 1024:1536])
                nc.scalar.dma_start(id2_sb[:], id2_d[:])
                nc.sync.dma_start(kts[3][:], kT_d[slot][:, 1536:2048])
                nc.scalar.dma_start(qts[6][:], qT_d[slot][:, 6 * CHUNK : 7 * CHUNK])
                for pc in range(2):
                    nc.scalar.dma_start(
                        vts[pc][:],
                        v_d[slot][:, pc * 8 * (HN + 1) : (pc + 1) * 8 * (HN + 1)],
                    )
                for pc in range(N_CHUNKS - 3, -1, -1):
                    nc.sync.dma_start(
                        qts[pc][:], qT_d[slot][:, pc * CHUNK : (pc + 1) * CHUNK]
                    )
                kslice = lambda j: kts[j // 4][:, (j % 4) * P : (j % 4 + 1) * P]
                qchunk = lambda ci: qts[ci][:]
                vslice = lambda j: vts[j // 8][
                    :, (j % 8) * (HN + 1) : (j % 8 + 1) * (HN + 1)
                ]
            else:
                kt = qk_pool.tile([P, SK], F16, tag="k")
                nc.sync.dma_start(kt[:], kT_d[slot])
                qt = qk_pool.tile([P, SQ], F16, tag="q")
                nc.sync.dma_start(qt[:], qT_d[slot])
                vt = v_pool.tile([P, N_SK_TILES * (HN + 1)], F16, tag="v")
                nc.sync.dma_start(vt[:], v_d[slot])
                kslice = lambda j: kt[:, j * P : (j + 1) * P]
                qchunk = lambda ci: qt[:, ci * CHUNK : (ci + 1) * CHUNK]
                vslice = lambda j: vt[:, j * (HN + 1) : (j + 1) * (HN + 1)]
            return kslice, qchunk, vslice

        # ---- score packer (within a slot): QK blocks stream into shared
        # PSUM groups; at flush, consecutive same-engine chunk spans are
        # exp'd by one op each (ACT exp, or DVE exp2 bit-trick for chunks
        # in DVE_CHUNKS); flushed at slot boundaries
        CAP = GROUP * CHUNK
        etmap = {}
        packer = {"sc": None, "fill": 0, "entries": []}

        def flush_packer():
            if packer["sc"] is None or packer["fill"] == 0:
                return
            fill = packer["fill"]
            et = e_pool.tile([P, CAP], F16, tag="expT", name="et")
            # group entries into consecutive same-engine spans; the 128-col
            # pad after each diagonal block breaks adjacency so pads are
            # never exp'd (they are never written or read)
            spans = []   # (engine, start, end)
            for (slot, ci, j), off, w in packer["entries"]:
                eng = "dve" if ci in DVE_CHUNKS else "act"
                if spans and spans[-1][0] == eng and spans[-1][2] == off:
                    spans[-1][2] = off + w
                else:
                    spans.append([eng, off, off + w])
            for eng, s, e in sorted(spans, key=lambda x: x[0] != "dve"):
                if eng == "dve":
                    nc.vector.tensor_scalar(
                        et.bitcast(I16)[:, s:e],
                        packer["sc"][:, s:e],
                        C_MUL, B_ADD,
                        mybir.AluOpType.mult, mybir.AluOpType.add,
                    )
                else:
                    nc.scalar.activation(
                        et[:, s:e], packer["sc"][:, s:e],
                        mybir.ActivationFunctionType.Exp,
                        scale=INV_NORM,
                    )
            for key, off, w in packer["entries"]:
                etmap[key] = (et, off)
            packer["sc"] = None
            packer["fill"] = 0
            packer["entries"] = []

        def emit_qk(slot, slot_io, ci):
            kslice, qchunk, _ = slot_io
            diag = 2 * ci + 1
            for j in range(2 * ci + 2):      # ascending; diagonal j last
                w = P if j == diag else CHUNK
                w_pad = CHUNK                  # diag padded to 256 so every
                                               # block stays 256-aligned and no
                                               # matmul crosses a PSUM bank
                if packer["sc"] is None or packer["fill"] + w_pad > CAP:
                    flush_packer()
                if packer["sc"] is None:
                    packer["sc"] = sc_ps.tile(
                        [P, CAP], F32, tag="scores", name="sc"
                    )
                sc, co = packer["sc"], packer["fill"]
                nc.tensor.matmul(
                    sc[:, co : co + w], kslice(j), qchunk(ci)[:, 0:w],
                    start=True, stop=True,
                )
                # causal mask on the PE: sc[m, n] += triT[n%128, m].
                # Must directly follow its QK matmul — start=False
                # continues only the most recent accumulation group.
                if j == diag:
                    nc.tensor.matmul(
                        sc[:, co : co + P], triT_sb[:], id2_sb[:, 0:P],
                        start=False, stop=True, skip_group_check=True,
                    )
                elif j == diag - 1:
                    nc.tensor.matmul(
                        sc[:, co + P : co + 2 * P], triT_sb[:], id2_sb[:, 0:P],
                        start=False, stop=True, skip_group_check=True,
                    )
                packer["entries"].append(((slot, ci, j), co, w))
                packer["fill"] = co + w_pad

        # ---- emit one chunk's PV + normalize + (maybe) out DMA ----------
        def emit_pv(slot, slot_io, ci, oq_tiles, done_quarters):
            _, _, vslice = slot_io
            exp_tiles = {j: etmap[(slot, ci, j)] for j in range(2 * ci + 2)}
            # one PSUM tile holds both context vectors of the chunk:
            # i_lo at cols [0,129), i_hi at cols [129,258)
            cx = cx_ps.tile([P, 2 * (HN + 1)], F32, tag="ctx")
            for i in (2 * ci + 1, 2 * ci):   # i_hi (first half of chunk), i_lo
                off = 0 if i == 2 * ci + 1 else P
                base = (HN + 1) if i == 2 * ci + 1 else 0
                pv_js = list(range(i + 1))
                for idx, j in enumerate(pv_js):
                    et, co = exp_tiles[j]
                    nc.tensor.matmul(
                        cx[:, base : base + HN + 1],
                        et[:, co + off : co + off + P], vslice(j),
                        start=(idx == 0), stop=(idx == len(pv_js) - 1),
                    )
            recip = r_pool.tile([P, 2], F32, tag="recip")
            nc.vector.reciprocal(
                recip[:], cx[:, HN : 2 * HN + 2 : HN + 1]
            )
            qt_idx = (2 * ci) // 4
            if qt_idx not in oq_tiles:
                oq_tiles[qt_idx] = o_pool.tile(
                    [P, 4 * HN], F16, tag="oq", name="oq"
                )
            ot = oq_tiles[qt_idx]
            col = (2 * ci % 4) * HN          # i_lo column; i_hi is the next one
            nc.vector.tensor_mul(
                ot[:, col : col + 2 * HN].rearrange("p (s c) -> p s c", s=2),
                cx[:].rearrange("p (s c) -> p s c", s=2)[:, :, 0:HN],
                recip[:].rearrange("p (s c) -> p s c", c=1).broadcast_to(
                    [P, 2, HN]
                ),
            )
            if slot == SLOTS_PER_CORE - 1:
                # last slot: ship each chunk's half-quarter as soon as it is
                # normalized, alternating queues so the final DMAs overlap
                h = ci % 2
                eng = nc.sync if ci % 2 == 0 else nc.scalar
                eng.dma_start(
                    out_d[slot, qt_idx][:, h * 2 * HN : (h + 1) * 2 * HN],
                    ot[:, h * 2 * HN : (h + 1) * 2 * HN],
                )
            else:
                done_quarters.setdefault(qt_idx, set()).add(ci)
                if len(done_quarters[qt_idx]) == 2:
                    nc.sync.dma_start(out_d[slot, qt_idx], oq_tiles[qt_idx][:])

        # ---- main schedule: PV runs as soon as its exp tiles exist ------
        pvq = []  # [(slot, slot_io, ci, oq_tiles, done_quarters)]

        def drain_pv(final=False):
            # keep one chunk pending (unless final) so PV trails the QK
            # stream; a chunk is ready once its diagonal block has been exp'd
            while pvq and (final or len(pvq) >= 2):
                slot, slot_io, ci, oq, dq = pvq[0]
                if (slot, ci, 2 * ci + 1) not in etmap:
                    return
                pvq.pop(0)
                emit_pv(slot, slot_io, ci, oq, dq)

        slot_state = {}
        for slot in range(SLOTS_PER_CORE):
            slot_io = load_slot(slot)
            slot_state[slot] = ({}, {})  # oq_tiles, done_quarters
            for ci in range(N_CHUNKS - 1, -1, -1):
                emit_qk(slot, slot_io, ci)
                oq, dq = slot_state[slot]
                pvq.append((slot, slot_io, ci, oq, dq))
                drain_pv()
            flush_packer()   # keep exp tiles slot-local
            drain_pv()
        drain_pv(final=True)
        assert not pvq

    nc.compile()
    return nc


_cache = {}


def _get_program(mask: np.ndarray):
    # this kernel is specialized to the standard causal mask
    m = np.asarray(mask)
    causal = np.triu(np.ones((SQ, SK), dtype=bool), k=1)
    for b in range(B):
        if not np.array_equal(m[b, 0], causal):
            raise ValueError("kernel specialized to causal attention mask")
    if "nc" not in _cache:
        _cache["nc"] = _build_program()
    return _cache["nc"]


def _core_slots(c):
    return [(0, 2 * c), (0, 2 * c + 1), (1, 2 * c), (1, 2 * c + 1)]


def prepare(query_layer, key_layer, value_layer, attention_mask):
    q = np.asarray(query_layer)
    k = np.asarray(key_layer)
    v = np.asarray(value_layer)
    nc = _get_program(np.asarray(attention_mask))

    # qT with the two 128-col tiles of each 256 chunk swapped:
    # sbuf layout col (256*ci + [0..255]) = sq (256*ci + [128..255, 0..127])
    q16 = q.astype(np.float16)                      # [SQ, B, NP, HN]
    qv = q16.reshape(N_CHUNKS, 2, P, B, NP, HN)[:, ::-1]   # swap tile pairs
    qT_all = np.ascontiguousarray(qv.transpose(3, 4, 5, 0, 1, 2)).reshape(
        B, NP, HN, SQ
    )
    k16 = k.astype(np.float16)
    kT_all = np.ascontiguousarray(k16.transpose(1, 2, 3, 0))  # [B, NP, HN, SK]

    v5 = v.reshape(N_SK_TILES, P, B, NP, HN).transpose(2, 3, 1, 0, 4)
    v_aug_all = np.empty((B, NP, P, N_SK_TILES, HN + 1), dtype=np.float16)
    v_aug_all[..., :HN] = v5
    v_aug_all[..., HN] = 1.0
    v_aug_all = v_aug_all.reshape(B, NP, P, N_SK_TILES * (HN + 1))

    # mask-matmul constants: sc[m, n] += sum_p triT[p, m] * ident2[p, n]
    #   = triT[n%128, m]  which must be NEG where (n%128) < m
    triT = np.where(
        np.arange(P)[:, None] < np.arange(P)[None, :], NEG, 0.0
    ).astype(np.float16)                            # triT[p, c] = NEG if p < c
    ident2 = np.concatenate([np.eye(P), np.eye(P)], axis=1).astype(np.float16)

    in_maps = []
    for c in range(N_CORES):
        slots = _core_slots(c)
        im = {
            "qT": np.ascontiguousarray(np.stack([qT_all[b, n] for b, n in slots])),
            "kT": np.ascontiguousarray(np.stack([kT_all[b, n] for b, n in slots])),
            "v_aug": np.ascontiguousarray(
                np.stack([v_aug_all[b, n] for b, n in slots])
            ),
            "triT": triT,
            "ident2": ident2,
        }
        in_maps.append(im)
    return nc, in_maps


def assemble(results):
    """Gather per-core 'out' arrays into the full [SQ, B, NP*HN] output."""
    full = np.empty((SQ, B, NP * HN), dtype=np.float32)
    for c in range(N_CORES):
        o = results[c]["out"]  # [4, 4, 128, 512] fp16
        for s, (b, n) in enumerate(_core_slots(c)):
            ctx = (
                o[s].reshape(4, P, 4, HN).transpose(0, 2, 1, 3).reshape(SQ, HN)
            )
            full[:, b, n * HN : (n + 1) * HN] = ctx.astype(np.float32)
    return full


def kernel(query_layer, key_layer, value_layer, attention_mask):
    from concourse.bass_utils import run_bass_kernel_spmd

    nc, in_maps = prepare(query_layer, key_layer, value_layer, attention_mask)
    res = run_bass_kernel_spmd(nc, in_maps, list(range(N_CORES)))
    return assemble(res.results)
